# revision 1
# baseline (speedup 1.0000x reference)
"""Trainium2 Bass kernel for nn_DynamicShortConvolution.

Reference computation (per token t, channel d):
    h    = silu(x @ w1)                       # [T, H]
    flat = h @ w2 + b2                        # [T, D*W]
    k    = flat.reshape(T, D, W)
    out[t, d] = silu(sum_w k[t, d, w] * x[t - (W-1) + w, d])

Sharding: 8 cores, each takes one (batch, half-of-T) shard of 2048 tokens
plus a 3-token left halo.  All per-core tensors are laid out TRANSPOSED
([D, T] with channels on SBUF partitions) so that:
  - the causal conv's token shift is a free-dim offset (no partition shifts)
  - mm1 consumes x.T directly, mm2 produces k.T directly
  - no on-device transposes at all; the host transposes shard I/O instead.
"""

import numpy as np

# Problem constants (hardcoded per harness contract).
B, T, D, H, W = 4, 4096, 2048, 256, 4
HALO = W - 1
N_CORES = 8
TOK = (B * T) // N_CORES  # tokens per core = 2048


def _build_nc(tok, d, h, tchunk, xstride, out_f32=True):
    """Build the single-core Bass/Tile program.

    tok: tokens per shard; d: channels; h: hidden; tchunk: token tile width
    (psum bank = 512 fp32 -> tchunk <= 512); xstride: per-dtile column stride
    of the xT sbuf tensor (>= tok + HALO, even for bf16 alignment).
    """
    import concourse.bass as bass
    import concourse.bacc as bacc
    import concourse.mybir as mybir
    import concourse.tile as tile

    f32 = mybir.dt.float32
    bf16 = mybir.dt.bfloat16
    AF = mybir.ActivationFunctionType
    ALU = mybir.AluOpType

    n_dt = d // 128        # d tiles
    n_hc = h // 128        # h tiles
    n_tc = tok // tchunk   # token chunks

    nc = bacc.Bacc()

    # DRAM I/O (host-prepared layouts)
    xT = nc.declare_dram_parameter("xT", [n_dt, 128, xstride], bf16, isOutput=False)
    w1 = nc.declare_dram_parameter("w1", [n_dt, 128, h], bf16, isOutput=False)
    # w2r[hc, hl, w, d] = w2[hc*128+hl, d*W + w]
    w2r = nc.declare_dram_parameter("w2r", [n_hc, 128, W, d], bf16, isOutput=False)
    # b2r[dt, p, w] = b2[(dt*128+p)*W + w]
    b2r = nc.declare_dram_parameter("b2r", [n_dt, 128, W], f32, isOutput=False)
    out_dt = f32 if out_f32 else bf16
    outT = nc.declare_dram_parameter("outT", [n_dt, 128, tok], out_dt, isOutput=True)

    with tile.TileContext(nc) as tc:
        with (
            tc.tile_pool(name="resident", bufs=1) as rpool,
            tc.tile_pool(name="work", bufs=3) as wpool,
            tc.tile_pool(name="psum", bufs=4, space="PSUM") as ppool,
        ):
            # ---- resident loads ----
            xT_sb = rpool.tile([128, n_dt * xstride], bf16, tag="xT")
            for dt in range(n_dt):
                nc.sync.dma_start(
                    xT_sb[:, dt * xstride:(dt + 1) * xstride], xT[dt])
            w1_sb = rpool.tile([128, n_dt * h], bf16, tag="w1")
            for dt in range(n_dt):
                nc.sync.dma_start(w1_sb[:, dt * h:(dt + 1) * h], w1[dt])
            w2_sb = rpool.tile([128, n_hc * W * d], bf16, tag="w2")
            for hc in range(n_hc):
                nc.sync.dma_start(
                    w2_sb[:, hc * W * d:(hc + 1) * W * d], w2r[hc])
            b2_sb = rpool.tile([128, n_dt * W], f32, tag="b2")
            for dt in range(n_dt):
                nc.sync.dma_start(b2_sb[:, dt * W:(dt + 1) * W], b2r[dt])
            # h.T, [h partitions x tok], bf16, hc-major in free dim
            hT_sb = rpool.tile([128, n_hc * tok], bf16, tag="hT")

            def x_slice(dt, col, n):
                return xT_sb[:, dt * xstride + col: dt * xstride + col + n]

            # ---- mm1: hT = silu(w1.T @ xT) ----
            for hc in range(n_hc):
                for tc_i in range(n_tc):
                    ph = ppool.tile([128, 2 * tchunk], f32, tag="ps")
                    for dt in range(n_dt):
                        nc.tensor.matmul(
                            ph[:, :tchunk],
                            w1_sb[:, dt * h + hc * 128: dt * h + hc * 128 + 128],
                            x_slice(dt, HALO + tc_i * tchunk, tchunk),
                            start=(dt == 0), stop=(dt == n_dt - 1),
                        )
                    nc.scalar.activation(
                        hT_sb[:, hc * tok + tc_i * tchunk:
                              hc * tok + (tc_i + 1) * tchunk],
                        ph[:, :tchunk], AF.Silu)

            # ---- mm2 + conv + silu ----
            # Process tchunk PAIRS (FD = 2*tchunk) so elementwise ops amortize
            # per-op overhead; one [128, 2*tchunk] psum tile per w (2 banks),
            # 4 slots in the pool.
            P = 2 * tchunk
            n_pi = n_tc // 2
            for dt in range(n_dt):
                for pi in range(n_pi):
                    j0 = pi * P
                    kws = []
                    for w in range(W):
                        kw = ppool.tile([128, P], f32, tag="ps")
                        for tc_j in range(2):
                            for hc in range(n_hc):
                                nc.tensor.matmul(
                                    kw[:, tc_j * tchunk:(tc_j + 1) * tchunk],
                                    w2_sb[:, hc * W * d + w * d + dt * 128:
                                          hc * W * d + w * d + dt * 128 + 128],
                                    hT_sb[:, hc * tok + j0 + tc_j * tchunk:
                                          hc * tok + j0 + (tc_j + 1) * tchunk],
                                    start=(hc == 0), stop=(hc == n_hc - 1),
                                )
                        kws.append(kw)
                    bias = [b2_sb[:, dt * W + w: dt * W + w + 1] for w in range(W)]
                    xw = [x_slice(dt, j0 + w, P) for w in range(W)]

                    m_all = wpool.tile([128, 4 * P], bf16, tag="mall")
                    # DVE fused evac+bias+mul for all four taps
                    for w in range(W):
                        nc.vector.scalar_tensor_tensor(
                            m_all[:, w * P:(w + 1) * P], kws[w][:], bias[w],
                            xw[w], op0=ALU.add, op1=ALU.mult)
                    # wide pairwise adds on GPSIMD
                    a2 = wpool.tile([128, 2 * P], bf16, tag="a2")
                    nc.gpsimd.tensor_add(
                        a2[:], m_all[:, :2 * P], m_all[:, 2 * P:])
                    acc = wpool.tile([128, P], bf16, tag="acc")
                    nc.gpsimd.tensor_add(acc[:], a2[:, :P], a2[:, P:])
                    # ACT: final silu
                    ot = wpool.tile([128, P], out_dt, tag="ot")
                    nc.scalar.activation(ot[:], acc[:], AF.Silu)
                    nc.sync.dma_start(outT[dt, :, j0:j0 + P], ot[:])
    nc.compile()
    return nc


def _prep_shards(x, w1, w2, b2, tok, d, h, halo, xstride):
    """Host-side shard prep. Returns list of per-core in_maps."""
    import ml_dtypes
    bf16 = ml_dtypes.bfloat16

    n_dt = d // 128
    n_hc = h // 128
    b, t, _ = x.shape
    shards_per_batch = (b * t // tok) // b
    w1_r = np.ascontiguousarray(
        w1.reshape(n_dt, 128, h)).astype(bf16)
    w2_r = np.ascontiguousarray(
        w2.reshape(h, d, W).transpose(0, 2, 1)  # [h, w, d]
        .reshape(n_hc, 128, W, d)).astype(bf16)
    b2_r = np.ascontiguousarray(b2.reshape(n_dt, 128, W)).astype(np.float32)

    in_maps = []
    for core in range(N_CORES):
        bi, half = divmod(core, shards_per_batch)
        t0 = half * tok
        xh = np.zeros((tok + halo, d), np.float32)
        lo = max(t0 - halo, 0)
        xh[halo - (t0 - lo):] = x[bi, lo: t0 + tok]
        xTc = np.zeros((n_dt, 128, xstride), bf16)
        xTc[:, :, : tok + halo] = (
            xh.T.astype(bf16).reshape(n_dt, 128, tok + halo))
        in_maps.append({
            "xT": xTc, "w1": w1_r, "w2r": w2_r, "b2r": b2_r})
    return in_maps


_NC_CACHE = {}


def kernel(x, w1, w2, b2, trace=False):
    from concourse.bass_utils import run_bass_kernel_spmd

    tok, d, h = TOK, D, H
    xstride = tok + HALO + 1  # even -> keeps bf16 4B alignment per dtile
    key = (tok, d, h)
    if key not in _NC_CACHE:
        _NC_CACHE[key] = _build_nc(tok, d, h, tchunk=512, xstride=xstride, out_f32=False)
    nc = _NC_CACHE[key]

    in_maps = _prep_shards(
        np.asarray(x, np.float32), np.asarray(w1, np.float32),
        np.asarray(w2, np.float32), np.asarray(b2, np.float32),
        tok, d, h, HALO, xstride)

    res = run_bass_kernel_spmd(nc, in_maps, core_ids=list(range(N_CORES)),
                               trace=trace)
    kernel.last_result = res

    shards_per_batch = (B * T // tok) // B
    out = np.empty((B, T, D), np.float32)
    for core in range(N_CORES):
        bi, half = divmod(core, shards_per_batch)
        oT = res.results[core]["outT"]  # [n_dt, 128, tok]
        out[bi, half * tok:(half + 1) * tok] = (
            oT.reshape(d, tok).T.astype(np.float32))
    return out

